# revision 45
# baseline (speedup 1.0000x reference)
"""GLOW coupling-flow (FrEIA-style) forward pass on 8 TRN2 NeuronCores.

Problem: B=8192, D=1024, C=512, H=512, L=8 coupling layers, each:
    xp = x[:, perm_k];  x1, x2 = xp[:, :512], xp[:, 512:]
    r2 = relu([x2, cond] @ w1 + b1) @ w2 + b2          (subnet 2)
    ls2 = 0.636*atan(r2[:, :512]);  y1 = exp(ls2)*x1 + r2[:, 512:]
    r1 = relu([y1, cond] @ w1 + b1) @ w2 + b2          (subnet 1)
    ls1 = 0.636*atan(r1[:, :512]);  y2 = exp(ls1)*x2 + r1[:, 512:]
    jac += sum(ls1 + ls2, axis=1);  x = [y1, y2]

Strategy:
- Pure data parallel: batch sharded 1024 rows/core, no collectives.
- Activations transposed (features on partitions, batch on free axis):
  weight-stationary matmuls  out^T = W^T @ inp^T  with K on partitions.
- fp16 matmuls (1 cyc/row, FWL weight loads, 2-byte traffic; end-to-end
  rel err ~3e-3 vs fp32 reference, gate is 2e-2).
- Column permutation per layer via DRAM round-trip + one dma_gather per
  (layer, half-batch pair); X stored pair-major [2, D, 512] so per-pair
  writes/gathers are contiguous ranges and pipeline across layers.
- Batch processed as 4 chunks (2 pairs) so gathers/epilogues of one pair
  hide behind compute of the other.
- ScalarE stays in the sigmoid_and_others table set for relu/atan; Exp
  (exp_and_others) is batched per pair to bound table switches.
- relu and the t-half (bias + add) on DVE; jac = ones-matmul partition
  sums of atan tiles accumulated in PSUM across the whole kernel.
"""
import sys

sys.path.insert(0, "/opt/trn_rl_repo")
import numpy as np

import concourse.bass as bass
import concourse.bacc as bacc
import concourse.tile as tile
import concourse.mybir as mybir
from concourse.tile import add_dep_helper
from concourse.bass_utils import run_bass_kernel_spmd

F32 = mybir.dt.float32
F16 = mybir.dt.float16
I16 = mybir.dt.int16
AF = mybir.ActivationFunctionType
ALU = mybir.AluOpType

B, D, C, H, L = 8192, 1024, 512, 512, 8
S = D // 2
ATAN_SCALE = 0.636
NCORE = 8
BS = B // NCORE          # 1024 batch rows per core
NCHUNK = 4
BC = BS // NCHUNK        # 256 batch columns per chunk
NPAIR = 2
PW = 2 * BC              # 512 batch columns per pair
KT_X = D // 128          # 8
KT_S = S // 128          # 4
P = 128

_NC_CACHE = {}


def build():
    nc = bacc.Bacc("TRN2", target_bir_lowering=False, num_swdge_queues=2)

    # X in DRAM is pair-major [NPAIR, D, PW]: per-pair writes, gathers, and
    # their dependencies are contiguous, non-overlapping ranges.
    x_in = nc.dram_tensor("x", [NPAIR, D, PW], F16, kind="ExternalInput")
    cond_in = nc.dram_tensor("cond", [C, BS], F16, kind="ExternalInput")
    # per (layer, subnet): [128, 8*512 (w1, kt-major) + 4*1024 (w2)]
    wts = nc.dram_tensor("wts", [L, 2, P, 8192], F16, kind="ExternalInput")
    # per (layer, subnet) 12 cols: b1 (4 mt) then b2 (8 mt)
    bias_in = nc.dram_tensor("bias", [P, L * 2 * 12], F32, kind="ExternalInput")
    gidx_in = nc.dram_tensor("gidx", [P, L * 64], I16, kind="ExternalInput")
    ones_in = nc.dram_tensor("ones", [P, 1], F16, kind="ExternalInput")
    out_x = nc.dram_tensor("out", [NPAIR, D, PW], F16, kind="ExternalOutput")
    out_j = nc.dram_tensor("jac", [1, BS], F32, kind="ExternalOutput")
    xb = [nc.dram_tensor(f"xbuf{i}", [NPAIR, D, PW], F16) for i in range(2)]

    def pair_T(t, pr, half=None):
        # view pair pr of a [NPAIR, D, PW] DRAM tensor as [128, kt, PW];
        # half 0 -> rows 0..511 (y1), half 1 -> rows 512..1023 (y2)
        if half is None:
            return t[pr].rearrange("(kt p) n -> p kt n", p=P)
        return t[pr][half * S:(half + 1) * S, :].rearrange(
            "(kt p) n -> p kt n", p=P)

    with tile.TileContext(nc) as tc:
        with (
            tc.tile_pool(name="const", bufs=1) as cpool,
            tc.tile_pool(name="wt", bufs=3) as wpool,
            tc.tile_pool(name="hp", bufs=3) as hpool,
            tc.tile_pool(name="aep", bufs=3) as aepool,
            tc.tile_pool(name="ab", bufs=6) as abpool,
            tc.tile_pool(name="brg", bufs=2) as brgpool,
            tc.tile_pool(name="ps", bufs=6, space="PSUM") as pspool,
            tc.tile_pool(name="psj", bufs=1, space="PSUM") as psjpool,
        ):
            # --- persistent loads (gather indices first: the layer-0
            # gathers are the longest startup pole) ---
            gsb = cpool.tile([P, L * 64], I16, tag="gidx")
            nc.sync.dma_start(gsb[:], gidx_in[:])
            ct = cpool.tile([P, KT_S, BS], F16, tag="cond")
            nc.sync.dma_start(ct[:], cond_in.rearrange("(kt p) n -> p kt n", p=P))
            bsb = cpool.tile([P, L * 2 * 12], F32, tag="bias")
            nc.sync.dma_start(bsb[:], bias_in[:])
            ones_t = cpool.tile([P, 1], F16, tag="ones")
            nc.sync.dma_start(ones_t[:], ones_in[:])
            z0 = cpool.tile([P, 1], F32, tag="z0")
            nc.gpsimd.memset(z0[:], 0.0)

            jac_ps = psjpool.tile([1, BS], F32, tag="jac")
            jac_started = [False] * NPAIR

            ab = None
            for k in range(L):
                dst = out_x if k == L - 1 else xb[k % 2]

                w_t = []
                for s in range(2):
                    w = wpool.tile([P, 8192], F16, tag="w")
                    nc.sync.dma_start(w[:], wts[k, s])
                    w_t.append(w)

                if k == 0:
                    # plain gathers from the input (data already present)
                    ab = []
                    marks = [None, None]
                    for pr in range(NPAIR):
                        t = abpool.tile([P, KT_X, PW], F16, tag="ab")
                        nc.gpsimd.dma_gather(
                            out_ap=t[:], in_ap=x_in[pr],
                            idxs_ap=gsb[:, k * 64:(k + 1) * 64],
                            num_idxs=BS, num_idxs_reg=BS,
                            elem_size=PW, elem_step=PW, queue_num=pr)
                        ab.append(t)

                # pre-generate next layer's gather descriptors now (the Q7
                # work overlaps this layer's compute); trigger_dma fires the
                # data DMA once this layer's y writes for the pair land.
                ab_next, gsems_next = None, None
                marks_next = [None, None]
                if k + 1 < L:
                    ab_next, gsems_next = [], []
                    for pr in range(NPAIR):
                        t = abpool.tile([P, KT_X, PW], F16, tag="ab")
                        gsem = nc.alloc_semaphore(f"gsem_{k + 1}_{pr}")
                        prep = nc.gpsimd.dma_gather(
                            out_ap=t[:], in_ap=dst[pr],
                            idxs_ap=gsb[:, (k + 1) * 64:(k + 2) * 64],
                            num_idxs=BS, num_idxs_reg=BS,
                            elem_size=PW, elem_step=PW, queue_num=pr,
                            prepare_only=True, sem=gsem)
                        # keep desc-gen behind the previous layer's marks on
                        # the Q7 stream so marks fire promptly
                        if marks[pr] is not None:
                            add_dep_helper(prep.ins, marks[pr], sync=False,
                                           reason="marks first")
                        ab_next.append(t)
                        gsems_next.append(gsem)

                # s=0 -> subnet2 (input x2 = ab[4:8], writes y1 into ab[0:4])
                # s=1 -> subnet1 (input y1 = ab[0:4], writes y2 into ab[4:8])
                for s in range(2):
                    bofs = (k * 2 + s) * 12
                    inp_lo = 4 * (1 - s)    # kt base of subnet input in ab
                    xm_lo = 4 * s           # kt base of mult operand in ab
                    for pair in range(NPAIR):
                        A = aepool.tile([P, KT_S, PW], F16, tag="A")
                        E = aepool.tile([P, KT_S, PW], F16, tag="E")
                        h = hpool.tile([P, KT_S, PW], F16, tag="h")
                        # mm1 per chunk (N=256; cond k-tiles first so the
                        # accumulation can start before the gather lands)
                        for half in range(2):
                            off = half * BC
                            cs = pair * PW + off
                            for mt in range(KT_S):
                                ph = pspool.tile([P, BC], F32, tag="ps")
                                kts = list(range(KT_S, KT_X)) + list(range(KT_S))
                                for i, kt in enumerate(kts):
                                    if kt < KT_S:
                                        rhs = ab[pair][:, inp_lo + kt,
                                                       off:off + BC]
                                    else:
                                        rhs = ct[:, kt - KT_S, cs:cs + BC]
                                    lo = kt * 512 + mt * P
                                    mm = nc.tensor.matmul(
                                        ph[:], w_t[s][:, lo:lo + P], rhs,
                                        start=(i == 0), stop=(i == KT_X - 1))
                                    if (s == 0 and i == KT_S
                                            and marks[pair] is not None):
                                        add_dep_helper(
                                            mm.ins, marks[pair], sync=True,
                                            reason="gather data landed")
                                nc.vector.tensor_scalar(
                                    out=h[:, mt, off:off + BC], in0=ph[:],
                                    scalar1=bsb[:, bofs + mt:bofs + mt + 1],
                                    scalar2=0.0, op0=ALU.add, op1=ALU.max)
                        # mm2 s-half at N=512 -> atan -> A
                        for mt in range(KT_S):
                            pr2 = pspool.tile([P, PW], F32, tag="ps")
                            for kt in range(KT_S):
                                lo = 4096 + kt * 1024 + mt * P
                                nc.tensor.matmul(
                                    pr2[:], w_t[s][:, lo:lo + P], h[:, kt, :],
                                    start=(kt == 0), stop=(kt == KT_S - 1))
                            nc.scalar.activation(
                                A[:, mt, :], pr2[:], AF.Arctan,
                                bias=bsb[:, bofs + 4 + mt:bofs + 5 + mt],
                                scale=1.0)
                        # exp for the whole pair (one table switch in)
                        nc.scalar.activation(E[:], A[:], AF.Exp,
                                             bias=z0[:, :1], scale=ATAN_SCALE)
                        # tmp = E * x_mult for the whole pair
                        tmp = aepool.tile([P, KT_S, PW], F16, tag="tmp")
                        mul = nc.vector.tensor_mul(
                            tmp[:], E[:], ab[pair][:, xm_lo:xm_lo + KT_S, :])
                        if marks[pair] is not None:
                            add_dep_helper(mul.ins, marks[pair], sync=True,
                                           reason="gather data landed")
                        # mm2 t-half at N=512 + y update
                        for mt in range(KT_S, 2 * KT_S):
                            pr2 = pspool.tile([P, PW], F32, tag="ps")
                            for kt in range(KT_S):
                                lo = 4096 + kt * 1024 + mt * P
                                nc.tensor.matmul(
                                    pr2[:], w_t[s][:, lo:lo + P], h[:, kt, :],
                                    start=(kt == 0), stop=(kt == KT_S - 1))
                            j = mt - KT_S
                            nc.vector.scalar_tensor_tensor(
                                out=ab[pair][:, xm_lo + j, :], in0=pr2[:],
                                scalar=bsb[:, bofs + 4 + mt:bofs + 5 + mt],
                                in1=tmp[:, j, :],
                                op0=ALU.add, op1=ALU.add)
                        # jac partition-sums over the pair (N=512)
                        pcs = pair * PW
                        for mt in range(KT_S):
                            nc.tensor.matmul(
                                jac_ps[:, pcs:pcs + PW], ones_t[:],
                                A[:, mt, :], start=not jac_started[pair],
                                stop=False, skip_group_check=True)
                            jac_started[pair] = True
                        if s == 1:
                            nc.sync.dma_start(pair_T(dst, pair), ab[pair][:])
                            if k + 1 < L:
                                b1 = brgpool.tile([1, 64], F16, tag="brg1")
                                nc.sync.dma_start(b1[:], dst[pair][0:1, 0:64])
                                b2 = brgpool.tile([1, 64], F16, tag="brg2")
                                gop = nc.gpsimd.tensor_copy(b2[:], b1[:])
                                trig = nc.gpsimd.trigger_dma(count=None,
                                                             queue_num=pair)
                                add_dep_helper(trig.ins, gop.ins, sync=False,
                                               reason="after bridge read")
                                wt_i = nc.gpsimd.wait_ge(gsems_next[pair], 16)
                                add_dep_helper(wt_i.ins, trig.ins, sync=False,
                                               reason="after trigger")
                                flag = brgpool.tile([1, 16], F16, tag="flag")
                                mark = nc.gpsimd.memset(flag[:], 0.0)
                                add_dep_helper(mark.ins, wt_i.ins, sync=False,
                                               reason="after wait")
                                marks_next[pair] = mark.ins

                ab = ab_next
                marks = marks_next

            jac_sb = cpool.tile([1, BS], F32, tag="jacsb")
            nc.scalar.mul(jac_sb[:], jac_ps[:], ATAN_SCALE)
            nc.sync.dma_start(out_j[:], jac_sb[:])

    nc.compile()
    return nc


def get_nc():
    if "nc" not in _NC_CACHE:
        _NC_CACHE["nc"] = build()
    return _NC_CACHE["nc"]


def _pack_weights(s1_w1, s1_b1, s1_w2, s1_b2, s2_w1, s2_b1, s2_w2, s2_b2):
    wts = np.empty((L, 2, P, 8192), dtype=np.float16)
    bias = np.empty((P, L * 2 * 12), dtype=np.float32)
    # s index 0 -> subnet2 (runs first), 1 -> subnet1
    for k in range(L):
        for s, (w1, b1, w2, b2) in enumerate(
            ((s2_w1, s2_b1, s2_w2, s2_b2), (s1_w1, s1_b1, s1_w2, s1_b2))
        ):
            wts[k, s, :, :4096] = (
                w1[k].reshape(8, P, 512).transpose(1, 0, 2).reshape(P, 4096))
            wts[k, s, :, 4096:] = (
                w2[k].reshape(4, P, 1024).transpose(1, 0, 2).reshape(P, 4096))
            bofs = (k * 2 + s) * 12
            bias[:, bofs:bofs + 4] = b1[k].reshape(4, P).T
            bias[:, bofs + 4:bofs + 12] = b2[k].reshape(8, P).T
    return wts, bias


def _run(inputs, trace=False):
    x = np.asarray(inputs["x"], dtype=np.float32)
    cond = np.asarray(inputs["cond"], dtype=np.float32)
    perms = np.asarray(inputs["perms"]).astype(np.int64)
    wts, bias = _pack_weights(
        np.asarray(inputs["s1_w1"], np.float32), np.asarray(inputs["s1_b1"], np.float32),
        np.asarray(inputs["s1_w2"], np.float32), np.asarray(inputs["s1_b2"], np.float32),
        np.asarray(inputs["s2_w1"], np.float32), np.asarray(inputs["s2_b1"], np.float32),
        np.asarray(inputs["s2_w2"], np.float32), np.asarray(inputs["s2_b2"], np.float32))

    # gather index tiles: 16-partition wrap, replicated across the 8 Q7 cores
    gidx = np.empty((P, L * 64), dtype=np.int16)
    for k in range(L):
        blk = np.zeros((16, 64), dtype=np.int16)
        pk = perms[k]
        for i in range(BS):
            blk[i % 16, i // 16] = pk[i]
        gidx[:, k * 64:(k + 1) * 64] = np.tile(blk, (8, 1))
    ones = np.ones((P, 1), dtype=np.float16)

    in_maps = []
    for ci in range(NCORE):
        xs = x[ci * BS:(ci + 1) * BS].T.astype(np.float16)          # [D, BS]
        xs = np.ascontiguousarray(
            xs.reshape(D, NPAIR, PW).transpose(1, 0, 2))            # [NPAIR, D, PW]
        cs = np.ascontiguousarray(cond[ci * BS:(ci + 1) * BS].T).astype(np.float16)
        in_maps.append(dict(x=xs, cond=cs, wts=wts, bias=bias,
                            gidx=gidx, ones=ones))

    nc = get_nc()
    res = run_bass_kernel_spmd(nc, in_maps, core_ids=list(range(NCORE)),
                               trace=trace)

    x_out = np.empty((B, D), dtype=np.float32)
    jac_out = np.empty((B,), dtype=np.float32)
    for ci in range(NCORE):
        oc = res.results[ci]["out"]                                 # [NPAIR, D, PW]
        x_out[ci * BS:(ci + 1) * BS] = (
            oc.transpose(1, 0, 2).reshape(D, BS).T.astype(np.float32))
        jac_out[ci * BS:(ci + 1) * BS] = res.results[ci]["jac"][0]
    return (x_out, jac_out), res


def kernel(**inputs):
    out, _ = _run(inputs, trace=False)
    return out


def kernel_traced(**inputs):
    return _run(inputs, trace=True)


# revision 46
# speedup vs baseline: 1.0065x; 1.0065x over previous
"""GLOW coupling-flow (FrEIA-style) forward pass on 8 TRN2 NeuronCores.

Problem: B=8192, D=1024, C=512, H=512, L=8 coupling layers, each:
    xp = x[:, perm_k];  x1, x2 = xp[:, :512], xp[:, 512:]
    r2 = relu([x2, cond] @ w1 + b1) @ w2 + b2          (subnet 2)
    ls2 = 0.636*atan(r2[:, :512]);  y1 = exp(ls2)*x1 + r2[:, 512:]
    r1 = relu([y1, cond] @ w1 + b1) @ w2 + b2          (subnet 1)
    ls1 = 0.636*atan(r1[:, :512]);  y2 = exp(ls1)*x2 + r1[:, 512:]
    jac += sum(ls1 + ls2, axis=1);  x = [y1, y2]

Strategy:
- Pure data parallel: batch sharded 1024 rows/core, no collectives.
- Activations transposed (features on partitions, batch on free axis):
  weight-stationary matmuls  out^T = W^T @ inp^T  with K on partitions.
- fp16 matmuls (1 cyc/row, FWL weight loads, 2-byte traffic; end-to-end
  rel err ~3e-3 vs fp32 reference, gate is 2e-2).
- Column permutation per layer via DRAM round-trip + one dma_gather per
  (layer, half-batch pair); X stored pair-major [2, D, 512] so per-pair
  writes/gathers are contiguous ranges and pipeline across layers.
- Batch processed as 4 chunks (2 pairs) so gathers/epilogues of one pair
  hide behind compute of the other.
- ScalarE stays in the sigmoid_and_others table set for relu/atan; Exp
  (exp_and_others) is batched per pair to bound table switches.
- relu and the t-half (bias + add) on DVE; jac = ones-matmul partition
  sums of atan tiles accumulated in PSUM across the whole kernel.
"""
import sys

sys.path.insert(0, "/opt/trn_rl_repo")
import numpy as np

import concourse.bass as bass
import concourse.bacc as bacc
import concourse.tile as tile
import concourse.mybir as mybir
from concourse.tile import add_dep_helper
from concourse.bass_utils import run_bass_kernel_spmd

F32 = mybir.dt.float32
F16 = mybir.dt.float16
I16 = mybir.dt.int16
AF = mybir.ActivationFunctionType
ALU = mybir.AluOpType

B, D, C, H, L = 8192, 1024, 512, 512, 8
S = D // 2
ATAN_SCALE = 0.636
NCORE = 8
BS = B // NCORE          # 1024 batch rows per core
NCHUNK = 4
BC = BS // NCHUNK        # 256 batch columns per chunk
NPAIR = 2
PW = 2 * BC              # 512 batch columns per pair
KT_X = D // 128          # 8
KT_S = S // 128          # 4
P = 128

_NC_CACHE = {}


def build():
    nc = bacc.Bacc("TRN2", target_bir_lowering=False, num_swdge_queues=2)

    # X in DRAM is pair-major [NPAIR, D, PW]: per-pair writes, gathers, and
    # their dependencies are contiguous, non-overlapping ranges.
    x_in = nc.dram_tensor("x", [NPAIR, D, PW], F16, kind="ExternalInput")
    cond_in = nc.dram_tensor("cond", [C, BS], F16, kind="ExternalInput")
    # per (layer, subnet): [128, 8*512 (w1, kt-major) + 4*1024 (w2)]
    wts = nc.dram_tensor("wts", [L, 2, P, 8192], F16, kind="ExternalInput")
    # per (layer, subnet) 12 cols: b1 (4 mt) then b2 (8 mt)
    bias_in = nc.dram_tensor("bias", [P, L * 2 * 12], F32, kind="ExternalInput")
    gidx_in = nc.dram_tensor("gidx", [P, L * 64], I16, kind="ExternalInput")
    ones_in = nc.dram_tensor("ones", [P, 1], F16, kind="ExternalInput")
    out_x = nc.dram_tensor("out", [NPAIR, D, PW], F16, kind="ExternalOutput")
    out_j = nc.dram_tensor("jac", [1, BS], F32, kind="ExternalOutput")
    xb = [nc.dram_tensor(f"xbuf{i}", [NPAIR, D, PW], F16) for i in range(2)]

    def pair_T(t, pr, half=None):
        # view pair pr of a [NPAIR, D, PW] DRAM tensor as [128, kt, PW];
        # half 0 -> rows 0..511 (y1), half 1 -> rows 512..1023 (y2)
        if half is None:
            return t[pr].rearrange("(kt p) n -> p kt n", p=P)
        return t[pr][half * S:(half + 1) * S, :].rearrange(
            "(kt p) n -> p kt n", p=P)

    with tile.TileContext(nc) as tc:
        with (
            tc.tile_pool(name="const", bufs=1) as cpool,
            tc.tile_pool(name="wt", bufs=3) as wpool,
            tc.tile_pool(name="hp", bufs=4) as hpool,
            tc.tile_pool(name="aep", bufs=4) as aepool,
            tc.tile_pool(name="ab", bufs=8) as abpool,
            tc.tile_pool(name="brg", bufs=2) as brgpool,
            tc.tile_pool(name="ps", bufs=6, space="PSUM") as pspool,
            tc.tile_pool(name="psj", bufs=1, space="PSUM") as psjpool,
        ):
            # --- persistent loads (gather indices first: the layer-0
            # gathers are the longest startup pole) ---
            gsb = cpool.tile([P, L * 64], I16, tag="gidx")
            nc.sync.dma_start(gsb[:], gidx_in[:])
            ct = cpool.tile([P, KT_S, BS], F16, tag="cond")
            nc.sync.dma_start(ct[:], cond_in.rearrange("(kt p) n -> p kt n", p=P))
            bsb = cpool.tile([P, L * 2 * 12], F32, tag="bias")
            nc.sync.dma_start(bsb[:], bias_in[:])
            ones_t = cpool.tile([P, 1], F16, tag="ones")
            nc.sync.dma_start(ones_t[:], ones_in[:])
            z0 = cpool.tile([P, 1], F32, tag="z0")
            nc.gpsimd.memset(z0[:], 0.0)

            jac_ps = psjpool.tile([1, BS], F32, tag="jac")
            jac_started = [False] * NPAIR

            ab = None
            for k in range(L):
                dst = out_x if k == L - 1 else xb[k % 2]

                w_t = []
                for s in range(2):
                    w = wpool.tile([P, 8192], F16, tag="w")
                    nc.sync.dma_start(w[:], wts[k, s])
                    w_t.append(w)

                if k == 0:
                    # plain gathers from the input (data already present)
                    ab = []
                    marks = [None, None]
                    for pr in range(NPAIR):
                        t = abpool.tile([P, KT_X, PW], F16, tag="ab")
                        nc.gpsimd.dma_gather(
                            out_ap=t[:], in_ap=x_in[pr],
                            idxs_ap=gsb[:, k * 64:(k + 1) * 64],
                            num_idxs=BS, num_idxs_reg=BS,
                            elem_size=PW, elem_step=PW, queue_num=pr)
                        ab.append(t)

                # pre-generate next layer's gather descriptors now (the Q7
                # work overlaps this layer's compute); trigger_dma fires the
                # data DMA once this layer's y writes for the pair land.
                ab_next, gsems_next = None, None
                marks_next = [None, None]
                if k + 1 < L:
                    ab_next, gsems_next = [], []
                    for pr in range(NPAIR):
                        t = abpool.tile([P, KT_X, PW], F16, tag="ab")
                        gsem = nc.alloc_semaphore(f"gsem_{k + 1}_{pr}")
                        prep = nc.gpsimd.dma_gather(
                            out_ap=t[:], in_ap=dst[pr],
                            idxs_ap=gsb[:, (k + 1) * 64:(k + 2) * 64],
                            num_idxs=BS, num_idxs_reg=BS,
                            elem_size=PW, elem_step=PW, queue_num=pr,
                            prepare_only=True, sem=gsem)
                        # keep desc-gen behind the previous layer's marks on
                        # the Q7 stream so marks fire promptly
                        if marks[pr] is not None:
                            add_dep_helper(prep.ins, marks[pr], sync=False,
                                           reason="marks first")
                        ab_next.append(t)
                        gsems_next.append(gsem)

                # s=0 -> subnet2 (input x2 = ab[4:8], writes y1 into ab[0:4])
                # s=1 -> subnet1 (input y1 = ab[0:4], writes y2 into ab[4:8])
                for s in range(2):
                    bofs = (k * 2 + s) * 12
                    inp_lo = 4 * (1 - s)    # kt base of subnet input in ab
                    xm_lo = 4 * s           # kt base of mult operand in ab
                    for pair in range(NPAIR):
                        A = aepool.tile([P, KT_S, PW], F16, tag="A")
                        E = aepool.tile([P, KT_S, PW], F16, tag="E")
                        h = hpool.tile([P, KT_S, PW], F16, tag="h")
                        # mm1 at N=512 (cond k-tiles first so the
                        # accumulation can start before the gather lands)
                        pcols = slice(pair * PW, (pair + 1) * PW)
                        for mt in range(KT_S):
                            ph = pspool.tile([P, PW], F32, tag="ps")
                            kts = list(range(KT_S, KT_X)) + list(range(KT_S))
                            for i, kt in enumerate(kts):
                                if kt < KT_S:
                                    rhs = ab[pair][:, inp_lo + kt, :]
                                else:
                                    rhs = ct[:, kt - KT_S, pcols]
                                lo = kt * 512 + mt * P
                                mm = nc.tensor.matmul(
                                    ph[:], w_t[s][:, lo:lo + P], rhs,
                                    start=(i == 0), stop=(i == KT_X - 1))
                                if (s == 0 and i == KT_S
                                        and marks[pair] is not None):
                                    add_dep_helper(
                                        mm.ins, marks[pair], sync=True,
                                        reason="gather data landed")
                            nc.vector.tensor_scalar(
                                out=h[:, mt, :], in0=ph[:],
                                scalar1=bsb[:, bofs + mt:bofs + mt + 1],
                                scalar2=0.0, op0=ALU.add, op1=ALU.max)
                        # mm2 s-half at N=512 -> atan -> A
                        for mt in range(KT_S):
                            pr2 = pspool.tile([P, PW], F32, tag="ps")
                            for kt in range(KT_S):
                                lo = 4096 + kt * 1024 + mt * P
                                nc.tensor.matmul(
                                    pr2[:], w_t[s][:, lo:lo + P], h[:, kt, :],
                                    start=(kt == 0), stop=(kt == KT_S - 1))
                            nc.scalar.activation(
                                A[:, mt, :], pr2[:], AF.Arctan,
                                bias=bsb[:, bofs + 4 + mt:bofs + 5 + mt],
                                scale=1.0)
                        # exp for the whole pair (one table switch in)
                        nc.scalar.activation(E[:], A[:], AF.Exp,
                                             bias=z0[:, :1], scale=ATAN_SCALE)
                        # tmp = E * x_mult for the whole pair
                        tmp = aepool.tile([P, KT_S, PW], F16, tag="tmp")
                        mul = nc.vector.tensor_mul(
                            tmp[:], E[:], ab[pair][:, xm_lo:xm_lo + KT_S, :])
                        if marks[pair] is not None:
                            add_dep_helper(mul.ins, marks[pair], sync=True,
                                           reason="gather data landed")
                        # mm2 t-half at N=512 + y update
                        for mt in range(KT_S, 2 * KT_S):
                            pr2 = pspool.tile([P, PW], F32, tag="ps")
                            for kt in range(KT_S):
                                lo = 4096 + kt * 1024 + mt * P
                                nc.tensor.matmul(
                                    pr2[:], w_t[s][:, lo:lo + P], h[:, kt, :],
                                    start=(kt == 0), stop=(kt == KT_S - 1))
                            j = mt - KT_S
                            nc.vector.scalar_tensor_tensor(
                                out=ab[pair][:, xm_lo + j, :], in0=pr2[:],
                                scalar=bsb[:, bofs + 4 + mt:bofs + 5 + mt],
                                in1=tmp[:, j, :],
                                op0=ALU.add, op1=ALU.add)
                        # jac: pre-sum A over mt on DVE (after the
                        # y-critical mul/stt), then one ones-matmul
                        # partition-sum accumulated in PSUM
                        pcs = pair * PW
                        As2 = aepool.tile([P, 2, PW], F16, tag="As2")
                        nc.vector.tensor_add(As2[:], A[:, 0:2, :], A[:, 2:4, :])
                        As = aepool.tile([P, PW], F16, tag="As")
                        nc.vector.tensor_add(As[:], As2[:, 0, :], As2[:, 1, :])
                        nc.tensor.matmul(
                            jac_ps[:, pcs:pcs + PW], ones_t[:], As[:],
                            start=not jac_started[pair],
                            stop=False, skip_group_check=True)
                        jac_started[pair] = True
                        if s == 1:
                            nc.sync.dma_start(pair_T(dst, pair), ab[pair][:])
                            if k + 1 < L:
                                b1 = brgpool.tile([1, 64], F16, tag="brg1")
                                nc.sync.dma_start(b1[:], dst[pair][0:1, 0:64])
                                b2 = brgpool.tile([1, 64], F16, tag="brg2")
                                gop = nc.gpsimd.tensor_copy(b2[:], b1[:])
                                trig = nc.gpsimd.trigger_dma(count=None,
                                                             queue_num=pair)
                                add_dep_helper(trig.ins, gop.ins, sync=False,
                                               reason="after bridge read")
                                wt_i = nc.gpsimd.wait_ge(gsems_next[pair], 16)
                                add_dep_helper(wt_i.ins, trig.ins, sync=False,
                                               reason="after trigger")
                                flag = brgpool.tile([1, 16], F16, tag="flag")
                                mark = nc.gpsimd.memset(flag[:], 0.0)
                                add_dep_helper(mark.ins, wt_i.ins, sync=False,
                                               reason="after wait")
                                marks_next[pair] = mark.ins

                ab = ab_next
                marks = marks_next

            jac_sb = cpool.tile([1, BS], F32, tag="jacsb")
            nc.scalar.mul(jac_sb[:], jac_ps[:], ATAN_SCALE)
            nc.sync.dma_start(out_j[:], jac_sb[:])

    nc.compile()
    return nc


def get_nc():
    if "nc" not in _NC_CACHE:
        _NC_CACHE["nc"] = build()
    return _NC_CACHE["nc"]


def _pack_weights(s1_w1, s1_b1, s1_w2, s1_b2, s2_w1, s2_b1, s2_w2, s2_b2):
    wts = np.empty((L, 2, P, 8192), dtype=np.float16)
    bias = np.empty((P, L * 2 * 12), dtype=np.float32)
    # s index 0 -> subnet2 (runs first), 1 -> subnet1
    for k in range(L):
        for s, (w1, b1, w2, b2) in enumerate(
            ((s2_w1, s2_b1, s2_w2, s2_b2), (s1_w1, s1_b1, s1_w2, s1_b2))
        ):
            wts[k, s, :, :4096] = (
                w1[k].reshape(8, P, 512).transpose(1, 0, 2).reshape(P, 4096))
            wts[k, s, :, 4096:] = (
                w2[k].reshape(4, P, 1024).transpose(1, 0, 2).reshape(P, 4096))
            bofs = (k * 2 + s) * 12
            bias[:, bofs:bofs + 4] = b1[k].reshape(4, P).T
            bias[:, bofs + 4:bofs + 12] = b2[k].reshape(8, P).T
    return wts, bias


def _run(inputs, trace=False):
    x = np.asarray(inputs["x"], dtype=np.float32)
    cond = np.asarray(inputs["cond"], dtype=np.float32)
    perms = np.asarray(inputs["perms"]).astype(np.int64)
    wts, bias = _pack_weights(
        np.asarray(inputs["s1_w1"], np.float32), np.asarray(inputs["s1_b1"], np.float32),
        np.asarray(inputs["s1_w2"], np.float32), np.asarray(inputs["s1_b2"], np.float32),
        np.asarray(inputs["s2_w1"], np.float32), np.asarray(inputs["s2_b1"], np.float32),
        np.asarray(inputs["s2_w2"], np.float32), np.asarray(inputs["s2_b2"], np.float32))

    # gather index tiles: 16-partition wrap, replicated across the 8 Q7 cores
    gidx = np.empty((P, L * 64), dtype=np.int16)
    for k in range(L):
        blk = np.zeros((16, 64), dtype=np.int16)
        pk = perms[k]
        for i in range(BS):
            blk[i % 16, i // 16] = pk[i]
        gidx[:, k * 64:(k + 1) * 64] = np.tile(blk, (8, 1))
    ones = np.ones((P, 1), dtype=np.float16)

    in_maps = []
    for ci in range(NCORE):
        xs = x[ci * BS:(ci + 1) * BS].T.astype(np.float16)          # [D, BS]
        xs = np.ascontiguousarray(
            xs.reshape(D, NPAIR, PW).transpose(1, 0, 2))            # [NPAIR, D, PW]
        cs = np.ascontiguousarray(cond[ci * BS:(ci + 1) * BS].T).astype(np.float16)
        in_maps.append(dict(x=xs, cond=cs, wts=wts, bias=bias,
                            gidx=gidx, ones=ones))

    nc = get_nc()
    res = run_bass_kernel_spmd(nc, in_maps, core_ids=list(range(NCORE)),
                               trace=trace)

    x_out = np.empty((B, D), dtype=np.float32)
    jac_out = np.empty((B,), dtype=np.float32)
    for ci in range(NCORE):
        oc = res.results[ci]["out"]                                 # [NPAIR, D, PW]
        x_out[ci * BS:(ci + 1) * BS] = (
            oc.transpose(1, 0, 2).reshape(D, BS).T.astype(np.float32))
        jac_out[ci * BS:(ci + 1) * BS] = res.results[ci]["jac"][0]
    return (x_out, jac_out), res


def kernel(**inputs):
    out, _ = _run(inputs, trace=False)
    return out


def kernel_traced(**inputs):
    return _run(inputs, trace=True)


# revision 47
# speedup vs baseline: 1.1078x; 1.1007x over previous
"""GLOW coupling-flow (FrEIA-style) forward pass on 8 TRN2 NeuronCores.

Problem: B=8192, D=1024, C=512, H=512, L=8 coupling layers, each:
    xp = x[:, perm_k];  x1, x2 = xp[:, :512], xp[:, 512:]
    r2 = relu([x2, cond] @ w1 + b1) @ w2 + b2          (subnet 2)
    ls2 = 0.636*atan(r2[:, :512]);  y1 = exp(ls2)*x1 + r2[:, 512:]
    r1 = relu([y1, cond] @ w1 + b1) @ w2 + b2          (subnet 1)
    ls1 = 0.636*atan(r1[:, :512]);  y2 = exp(ls1)*x2 + r1[:, 512:]
    jac += sum(ls1 + ls2, axis=1);  x = [y1, y2]

Strategy:
- Pure data parallel: batch sharded 1024 rows/core, no collectives.
- Activations transposed (features on partitions, batch on free axis):
  weight-stationary matmuls  out^T = W^T @ inp^T  with K on partitions.
- fp16 matmuls (1 cyc/row, FWL weight loads, 2-byte traffic; end-to-end
  rel err ~3e-3 vs fp32 reference, gate is 2e-2).
- Column permutation per layer via DRAM round-trip + one dma_gather per
  (layer, half-batch pair); X stored pair-major [2, D, 512] so per-pair
  writes/gathers are contiguous ranges and pipeline across layers.
- Batch processed as 4 chunks (2 pairs) so gathers/epilogues of one pair
  hide behind compute of the other.
- ScalarE stays in the sigmoid_and_others table set for relu/atan; Exp
  (exp_and_others) is batched per pair to bound table switches.
- relu and the t-half (bias + add) on DVE; jac = ones-matmul partition
  sums of atan tiles accumulated in PSUM across the whole kernel.
"""
import sys

sys.path.insert(0, "/opt/trn_rl_repo")
import numpy as np

import concourse.bass as bass
import concourse.bacc as bacc
import concourse.tile as tile
import concourse.mybir as mybir
from concourse.tile import add_dep_helper
from concourse.bass_utils import run_bass_kernel_spmd

F32 = mybir.dt.float32
F16 = mybir.dt.float16
I16 = mybir.dt.int16
AF = mybir.ActivationFunctionType
ALU = mybir.AluOpType

B, D, C, H, L = 8192, 1024, 512, 512, 8
S = D // 2
ATAN_SCALE = 0.636
NCORE = 8
BS = B // NCORE          # 1024 batch rows per core
NCHUNK = 4
BC = BS // NCHUNK        # 256 batch columns per chunk
NPAIR = 2
PW = 2 * BC              # 512 batch columns per pair
KT_X = D // 128          # 8
KT_S = S // 128          # 4
P = 128

_NC_CACHE = {}


def build():
    nc = bacc.Bacc("TRN2", target_bir_lowering=False, num_swdge_queues=2)

    # X in DRAM is pair-major [NPAIR, D, PW]: per-pair writes, gathers, and
    # their dependencies are contiguous, non-overlapping ranges.
    x_in = nc.dram_tensor("x", [NPAIR, D, PW], F16, kind="ExternalInput")
    cond_in = nc.dram_tensor("cond", [C, BS], F16, kind="ExternalInput")
    # per (layer, subnet): [128, 8*512 (w1, kt-major) + 4*1024 (w2)]
    wts = nc.dram_tensor("wts", [L, 2, P, 8192], F16, kind="ExternalInput")
    # per (layer, subnet) 12 cols: b1 (4 mt) then b2 (8 mt)
    bias_in = nc.dram_tensor("bias", [P, L * 2 * 12], F32, kind="ExternalInput")
    gidx_in = nc.dram_tensor("gidx", [P, L * 64], I16, kind="ExternalInput")
    ones_in = nc.dram_tensor("ones", [P, 1], F16, kind="ExternalInput")
    out_x = nc.dram_tensor("out", [NPAIR, D, PW], F16, kind="ExternalOutput")
    out_j = nc.dram_tensor("jac", [1, BS], F32, kind="ExternalOutput")
    xb = [nc.dram_tensor(f"xbuf{i}", [NPAIR, D, PW], F16) for i in range(2)]

    def pair_T(t, pr, half=None):
        # view pair pr of a [NPAIR, D, PW] DRAM tensor as [128, kt, PW];
        # half 0 -> rows 0..511 (y1), half 1 -> rows 512..1023 (y2)
        if half is None:
            return t[pr].rearrange("(kt p) n -> p kt n", p=P)
        return t[pr][half * S:(half + 1) * S, :].rearrange(
            "(kt p) n -> p kt n", p=P)

    with tile.TileContext(nc) as tc:
        with (
            tc.tile_pool(name="const", bufs=1) as cpool,
            tc.tile_pool(name="wt", bufs=3) as wpool,
            tc.tile_pool(name="hp", bufs=3) as hpool,
            tc.tile_pool(name="aep", bufs=3) as aepool,
            tc.tile_pool(name="ab", bufs=6) as abpool,
            tc.tile_pool(name="brg", bufs=2) as brgpool,
            tc.tile_pool(name="ps", bufs=6, space="PSUM") as pspool,
            tc.tile_pool(name="psj", bufs=1, space="PSUM") as psjpool,
        ):
            # --- persistent loads (gather indices first: the layer-0
            # gathers are the longest startup pole) ---
            gsb = cpool.tile([P, L * 64], I16, tag="gidx")
            nc.sync.dma_start(gsb[:], gidx_in[:])
            ct = cpool.tile([P, KT_S, BS], F16, tag="cond")
            nc.sync.dma_start(ct[:], cond_in.rearrange("(kt p) n -> p kt n", p=P))
            bsb = cpool.tile([P, L * 2 * 12], F32, tag="bias")
            nc.sync.dma_start(bsb[:], bias_in[:])
            ones_t = cpool.tile([P, 1], F16, tag="ones")
            nc.sync.dma_start(ones_t[:], ones_in[:])
            z0 = cpool.tile([P, 1], F32, tag="z0")
            nc.gpsimd.memset(z0[:], 0.0)

            jac_ps = psjpool.tile([1, BS], F32, tag="jac")
            jac_started = [False] * NPAIR

            ab = None
            for k in range(L):
                dst = out_x if k == L - 1 else xb[k % 2]

                w_t = []
                for s in range(2):
                    w = wpool.tile([P, 8192], F16, tag="w")
                    nc.sync.dma_start(w[:], wts[k, s])
                    w_t.append(w)

                if k == 0:
                    # plain gathers from the input (data already present)
                    ab = []
                    marks = [None, None]
                    for pr in range(NPAIR):
                        t = abpool.tile([P, KT_X, PW], F16, tag="ab")
                        nc.gpsimd.dma_gather(
                            out_ap=t[:], in_ap=x_in[pr],
                            idxs_ap=gsb[:, k * 64:(k + 1) * 64],
                            num_idxs=BS, num_idxs_reg=BS,
                            elem_size=PW, elem_step=PW, queue_num=pr)
                        ab.append(t)

                # pre-generate next layer's gather descriptors now (the Q7
                # work overlaps this layer's compute); trigger_dma fires the
                # data DMA once this layer's y writes for the pair land.
                ab_next, gsems_next = None, None
                marks_next = [None, None]
                if k + 1 < L:
                    ab_next, gsems_next = [], []
                    for pr in range(NPAIR):
                        t = abpool.tile([P, KT_X, PW], F16, tag="ab")
                        gsem = nc.alloc_semaphore(f"gsem_{k + 1}_{pr}")
                        prep = nc.gpsimd.dma_gather(
                            out_ap=t[:], in_ap=dst[pr],
                            idxs_ap=gsb[:, (k + 1) * 64:(k + 2) * 64],
                            num_idxs=BS, num_idxs_reg=BS,
                            elem_size=PW, elem_step=PW, queue_num=pr,
                            prepare_only=True, sem=gsem)
                        # keep desc-gen behind the previous layer's marks on
                        # the Q7 stream so marks fire promptly
                        if marks[pr] is not None:
                            add_dep_helper(prep.ins, marks[pr], sync=False,
                                           reason="marks first")
                        ab_next.append(t)
                        gsems_next.append(gsem)

                # s=0 -> subnet2 (input x2 = ab[4:8], writes y1 into ab[0:4])
                # s=1 -> subnet1 (input y1 = ab[0:4], writes y2 into ab[4:8])
                for s in range(2):
                    bofs = (k * 2 + s) * 12
                    inp_lo = 4 * (1 - s)    # kt base of subnet input in ab
                    xm_lo = 4 * s           # kt base of mult operand in ab
                    for pair in range(NPAIR):
                        A = aepool.tile([P, KT_S, PW], F16, tag="A")
                        E = aepool.tile([P, KT_S, PW], F16, tag="E")
                        h = hpool.tile([P, KT_S, PW], F16, tag="h")
                        # mm1 per chunk (N=256; cond k-tiles first so the
                        # accumulation can start before the gather lands)
                        for half in range(2):
                            off = half * BC
                            cs = pair * PW + off
                            for mt in range(KT_S):
                                ph = pspool.tile([P, BC], F32, tag="ps")
                                kts = list(range(KT_S, KT_X)) + list(range(KT_S))
                                for i, kt in enumerate(kts):
                                    if kt < KT_S:
                                        rhs = ab[pair][:, inp_lo + kt,
                                                       off:off + BC]
                                    else:
                                        rhs = ct[:, kt - KT_S, cs:cs + BC]
                                    lo = kt * 512 + mt * P
                                    mm = nc.tensor.matmul(
                                        ph[:], w_t[s][:, lo:lo + P], rhs,
                                        start=(i == 0), stop=(i == KT_X - 1))
                                    if (s == 0 and i == KT_S
                                            and marks[pair] is not None):
                                        add_dep_helper(
                                            mm.ins, marks[pair], sync=True,
                                            reason="gather data landed")
                                nc.vector.tensor_scalar(
                                    out=h[:, mt, off:off + BC], in0=ph[:],
                                    scalar1=bsb[:, bofs + mt:bofs + mt + 1],
                                    scalar2=0.0, op0=ALU.add, op1=ALU.max)
                        # mm2 s-half at N=512 -> atan -> A
                        for mt in range(KT_S):
                            pr2 = pspool.tile([P, PW], F32, tag="ps")
                            for kt in range(KT_S):
                                lo = 4096 + kt * 1024 + mt * P
                                nc.tensor.matmul(
                                    pr2[:], w_t[s][:, lo:lo + P], h[:, kt, :],
                                    start=(kt == 0), stop=(kt == KT_S - 1))
                            nc.scalar.activation(
                                A[:, mt, :], pr2[:], AF.Arctan,
                                bias=bsb[:, bofs + 4 + mt:bofs + 5 + mt],
                                scale=1.0)
                        # exp for the whole pair (one table switch in)
                        nc.scalar.activation(E[:], A[:], AF.Exp,
                                             bias=z0[:, :1], scale=ATAN_SCALE)
                        # tmp = E * x_mult for the whole pair
                        tmp = aepool.tile([P, KT_S, PW], F16, tag="tmp")
                        mul = nc.vector.tensor_mul(
                            tmp[:], E[:], ab[pair][:, xm_lo:xm_lo + KT_S, :])
                        if marks[pair] is not None:
                            add_dep_helper(mul.ins, marks[pair], sync=True,
                                           reason="gather data landed")
                        # mm2 t-half at N=512 + y update
                        for mt in range(KT_S, 2 * KT_S):
                            pr2 = pspool.tile([P, PW], F32, tag="ps")
                            for kt in range(KT_S):
                                lo = 4096 + kt * 1024 + mt * P
                                nc.tensor.matmul(
                                    pr2[:], w_t[s][:, lo:lo + P], h[:, kt, :],
                                    start=(kt == 0), stop=(kt == KT_S - 1))
                            j = mt - KT_S
                            nc.vector.scalar_tensor_tensor(
                                out=ab[pair][:, xm_lo + j, :], in0=pr2[:],
                                scalar=bsb[:, bofs + 4 + mt:bofs + 5 + mt],
                                in1=tmp[:, j, :],
                                op0=ALU.add, op1=ALU.add)
                        # jac partition-sums over the pair (N=512)
                        pcs = pair * PW
                        for mt in range(KT_S):
                            nc.tensor.matmul(
                                jac_ps[:, pcs:pcs + PW], ones_t[:],
                                A[:, mt, :], start=not jac_started[pair],
                                stop=False, skip_group_check=True)
                            jac_started[pair] = True
                        if s == 1:
                            nc.sync.dma_start(pair_T(dst, pair), ab[pair][:])
                            if k + 1 < L:
                                b1 = brgpool.tile([1, 64], F16, tag="brg1")
                                nc.sync.dma_start(b1[:], dst[pair][0:1, 0:64])
                                b2 = brgpool.tile([1, 64], F16, tag="brg2")
                                gop = nc.gpsimd.tensor_copy(b2[:], b1[:])
                                trig = nc.gpsimd.trigger_dma(count=None,
                                                             queue_num=pair)
                                add_dep_helper(trig.ins, gop.ins, sync=False,
                                               reason="after bridge read")
                                wt_i = nc.gpsimd.wait_ge(gsems_next[pair], 16)
                                add_dep_helper(wt_i.ins, trig.ins, sync=False,
                                               reason="after trigger")
                                flag = brgpool.tile([1, 16], F16, tag="flag")
                                mark = nc.gpsimd.memset(flag[:], 0.0)
                                add_dep_helper(mark.ins, wt_i.ins, sync=False,
                                               reason="after wait")
                                marks_next[pair] = mark.ins

                ab = ab_next
                marks = marks_next

            jac_sb = cpool.tile([1, BS], F32, tag="jacsb")
            nc.scalar.mul(jac_sb[:], jac_ps[:], ATAN_SCALE)
            nc.sync.dma_start(out_j[:], jac_sb[:])

    nc.compile()
    return nc


def get_nc():
    if "nc" not in _NC_CACHE:
        _NC_CACHE["nc"] = build()
    return _NC_CACHE["nc"]


def _pack_weights(s1_w1, s1_b1, s1_w2, s1_b2, s2_w1, s2_b1, s2_w2, s2_b2):
    wts = np.empty((L, 2, P, 8192), dtype=np.float16)
    bias = np.empty((P, L * 2 * 12), dtype=np.float32)
    # s index 0 -> subnet2 (runs first), 1 -> subnet1
    for k in range(L):
        for s, (w1, b1, w2, b2) in enumerate(
            ((s2_w1, s2_b1, s2_w2, s2_b2), (s1_w1, s1_b1, s1_w2, s1_b2))
        ):
            wts[k, s, :, :4096] = (
                w1[k].reshape(8, P, 512).transpose(1, 0, 2).reshape(P, 4096))
            wts[k, s, :, 4096:] = (
                w2[k].reshape(4, P, 1024).transpose(1, 0, 2).reshape(P, 4096))
            bofs = (k * 2 + s) * 12
            bias[:, bofs:bofs + 4] = b1[k].reshape(4, P).T
            bias[:, bofs + 4:bofs + 12] = b2[k].reshape(8, P).T
    return wts, bias


def _run(inputs, trace=False):
    x = np.asarray(inputs["x"], dtype=np.float32)
    cond = np.asarray(inputs["cond"], dtype=np.float32)
    perms = np.asarray(inputs["perms"]).astype(np.int64)
    wts, bias = _pack_weights(
        np.asarray(inputs["s1_w1"], np.float32), np.asarray(inputs["s1_b1"], np.float32),
        np.asarray(inputs["s1_w2"], np.float32), np.asarray(inputs["s1_b2"], np.float32),
        np.asarray(inputs["s2_w1"], np.float32), np.asarray(inputs["s2_b1"], np.float32),
        np.asarray(inputs["s2_w2"], np.float32), np.asarray(inputs["s2_b2"], np.float32))

    # gather index tiles: 16-partition wrap, replicated across the 8 Q7 cores
    gidx = np.empty((P, L * 64), dtype=np.int16)
    for k in range(L):
        blk = np.zeros((16, 64), dtype=np.int16)
        pk = perms[k]
        for i in range(BS):
            blk[i % 16, i // 16] = pk[i]
        gidx[:, k * 64:(k + 1) * 64] = np.tile(blk, (8, 1))
    ones = np.ones((P, 1), dtype=np.float16)

    in_maps = []
    for ci in range(NCORE):
        xs = x[ci * BS:(ci + 1) * BS].T.astype(np.float16)          # [D, BS]
        xs = np.ascontiguousarray(
            xs.reshape(D, NPAIR, PW).transpose(1, 0, 2))            # [NPAIR, D, PW]
        cs = np.ascontiguousarray(cond[ci * BS:(ci + 1) * BS].T).astype(np.float16)
        in_maps.append(dict(x=xs, cond=cs, wts=wts, bias=bias,
                            gidx=gidx, ones=ones))

    nc = get_nc()
    res = run_bass_kernel_spmd(nc, in_maps, core_ids=list(range(NCORE)),
                               trace=trace)

    x_out = np.empty((B, D), dtype=np.float32)
    jac_out = np.empty((B,), dtype=np.float32)
    for ci in range(NCORE):
        oc = res.results[ci]["out"]                                 # [NPAIR, D, PW]
        x_out[ci * BS:(ci + 1) * BS] = (
            oc.transpose(1, 0, 2).reshape(D, BS).T.astype(np.float32))
        jac_out[ci * BS:(ci + 1) * BS] = res.results[ci]["jac"][0]
    return (x_out, jac_out), res


def kernel(**inputs):
    out, _ = _run(inputs, trace=False)
    return out


def kernel_traced(**inputs):
    return _run(inputs, trace=True)


# revision 48
# speedup vs baseline: 1.1663x; 1.0527x over previous
"""GLOW coupling-flow (FrEIA-style) forward pass on 8 TRN2 NeuronCores.

Problem: B=8192, D=1024, C=512, H=512, L=8 coupling layers, each:
    xp = x[:, perm_k];  x1, x2 = xp[:, :512], xp[:, 512:]
    r2 = relu([x2, cond] @ w1 + b1) @ w2 + b2          (subnet 2)
    ls2 = 0.636*atan(r2[:, :512]);  y1 = exp(ls2)*x1 + r2[:, 512:]
    r1 = relu([y1, cond] @ w1 + b1) @ w2 + b2          (subnet 1)
    ls1 = 0.636*atan(r1[:, :512]);  y2 = exp(ls1)*x2 + r1[:, 512:]
    jac += sum(ls1 + ls2, axis=1);  x = [y1, y2]

Strategy:
- Pure data parallel: batch sharded 1024 rows/core, no collectives.
- Activations transposed (features on partitions, batch on free axis):
  weight-stationary matmuls  out^T = W^T @ inp^T  with K on partitions.
- fp16 matmuls (1 cyc/row, FWL weight loads, 2-byte traffic; end-to-end
  rel err ~3e-3 vs fp32 reference, gate is 2e-2).
- Column permutation per layer via DRAM round-trip + one dma_gather per
  (layer, half-batch pair); X stored pair-major [2, D, 512] so per-pair
  writes/gathers are contiguous ranges and pipeline across layers.
- Batch processed as 4 chunks (2 pairs) so gathers/epilogues of one pair
  hide behind compute of the other.
- ScalarE stays in the sigmoid_and_others table set for relu/atan; Exp
  (exp_and_others) is batched per pair to bound table switches.
- relu and the t-half (bias + add) on DVE; jac = ones-matmul partition
  sums of atan tiles accumulated in PSUM across the whole kernel.
"""
import sys

sys.path.insert(0, "/opt/trn_rl_repo")
import numpy as np

import concourse.bass as bass
import concourse.bacc as bacc
import concourse.tile as tile
import concourse.mybir as mybir
from concourse.tile import add_dep_helper
from concourse.bass_utils import run_bass_kernel_spmd

F32 = mybir.dt.float32
F16 = mybir.dt.float16
I16 = mybir.dt.int16
AF = mybir.ActivationFunctionType
ALU = mybir.AluOpType

B, D, C, H, L = 8192, 1024, 512, 512, 8
S = D // 2
ATAN_SCALE = 0.636
NCORE = 8
BS = B // NCORE          # 1024 batch rows per core
NCHUNK = 4
BC = BS // NCHUNK        # 256 batch columns per chunk
NPAIR = 2
PW = 2 * BC              # 512 batch columns per pair
KT_X = D // 128          # 8
KT_S = S // 128          # 4
P = 128

_NC_CACHE = {}


def build():
    nc = bacc.Bacc("TRN2", target_bir_lowering=False, num_swdge_queues=2)

    # X in DRAM is pair-major [NPAIR, D, PW]: per-pair writes, gathers, and
    # their dependencies are contiguous, non-overlapping ranges.
    x_in = nc.dram_tensor("x", [NPAIR, D, PW], F16, kind="ExternalInput")
    cond_in = nc.dram_tensor("cond", [C, BS], F16, kind="ExternalInput")
    # per (layer, subnet): [128, 8*512 (w1, kt-major) + 4*1024 (w2)]
    wts = nc.dram_tensor("wts", [L, 2, P, 8192], F16, kind="ExternalInput")
    # per (layer, subnet) 12 cols: b1 (4 mt) then b2 (8 mt)
    bias_in = nc.dram_tensor("bias", [P, L * 2 * 12], F32, kind="ExternalInput")
    gidx_in = nc.dram_tensor("gidx", [P, L * 64], I16, kind="ExternalInput")
    ones_in = nc.dram_tensor("ones", [P, 1], F16, kind="ExternalInput")
    out_x = nc.dram_tensor("out", [NPAIR, D, PW], F16, kind="ExternalOutput")
    out_j = nc.dram_tensor("jac", [1, BS], F32, kind="ExternalOutput")
    xb = [nc.dram_tensor(f"xbuf{i}", [NPAIR, D, PW], F16) for i in range(2)]

    def pair_T(t, pr, half=None):
        # view pair pr of a [NPAIR, D, PW] DRAM tensor as [128, kt, PW];
        # half 0 -> rows 0..511 (y1), half 1 -> rows 512..1023 (y2)
        if half is None:
            return t[pr].rearrange("(kt p) n -> p kt n", p=P)
        return t[pr][half * S:(half + 1) * S, :].rearrange(
            "(kt p) n -> p kt n", p=P)

    with tile.TileContext(nc) as tc:
        with (
            tc.tile_pool(name="const", bufs=1) as cpool,
            tc.tile_pool(name="wt", bufs=3) as wpool,
            tc.tile_pool(name="hp", bufs=3) as hpool,
            tc.tile_pool(name="aep", bufs=3) as aepool,
            tc.tile_pool(name="ab", bufs=6) as abpool,
            tc.tile_pool(name="brg", bufs=2) as brgpool,
            tc.tile_pool(name="ps", bufs=6, space="PSUM") as pspool,
            tc.tile_pool(name="psj", bufs=1, space="PSUM") as psjpool,
        ):
            # --- persistent loads (gather indices first: the layer-0
            # gathers are the longest startup pole) ---
            gsb = cpool.tile([P, L * 64], I16, tag="gidx")
            nc.sync.dma_start(gsb[:], gidx_in[:])
            ct = cpool.tile([P, KT_S, BS], F16, tag="cond")
            nc.sync.dma_start(ct[:], cond_in.rearrange("(kt p) n -> p kt n", p=P))
            bsb = cpool.tile([P, L * 2 * 12], F32, tag="bias")
            nc.sync.dma_start(bsb[:], bias_in[:])
            ones_t = cpool.tile([P, 1], F16, tag="ones")
            nc.sync.dma_start(ones_t[:], ones_in[:])
            z0 = cpool.tile([P, 1], F32, tag="z0")
            nc.gpsimd.memset(z0[:], 0.0)

            jac_ps = psjpool.tile([1, BS], F32, tag="jac")
            jac_started = [False] * NPAIR

            ab = None
            for k in range(L):
                dst = out_x if k == L - 1 else xb[k % 2]

                w_t = []
                for s in range(2):
                    w = wpool.tile([P, 8192], F16, tag="w")
                    nc.sync.dma_start(w[:], wts[k, s])
                    w_t.append(w)

                if k == 0:
                    # plain gathers from the input (data already present),
                    # split per half with the b-half (subnet-2's matmul
                    # input) first so TensorE starts after ~half the
                    # descriptor generation
                    ab = []
                    marks = [None, None]
                    for pr in range(NPAIR):
                        t = abpool.tile([P, KT_X, PW], F16, tag="ab")
                        for hf in (1, 0):
                            nc.gpsimd.dma_gather(
                                out_ap=t[:, hf * KT_S:(hf + 1) * KT_S, :],
                                in_ap=x_in[pr],
                                idxs_ap=gsb[:, k * 64 + hf * 32:
                                            k * 64 + (hf + 1) * 32],
                                num_idxs=S, num_idxs_reg=S,
                                elem_size=PW, elem_step=PW, queue_num=pr)
                        ab.append(t)

                # pre-generate next layer's gather descriptors now (the Q7
                # work overlaps this layer's compute); trigger_dma fires the
                # data DMA once this layer's y writes for the pair land.
                ab_next, gsems_next = None, None
                marks_next = [None, None]
                if k + 1 < L:
                    ab_next, gsems_next = [], []
                    for pr in range(NPAIR):
                        t = abpool.tile([P, KT_X, PW], F16, tag="ab")
                        gsem = nc.alloc_semaphore(f"gsem_{k + 1}_{pr}")
                        prep = nc.gpsimd.dma_gather(
                            out_ap=t[:], in_ap=dst[pr],
                            idxs_ap=gsb[:, (k + 1) * 64:(k + 2) * 64],
                            num_idxs=BS, num_idxs_reg=BS,
                            elem_size=PW, elem_step=PW, queue_num=pr,
                            prepare_only=True, sem=gsem)
                        # keep desc-gen behind the previous layer's marks on
                        # the Q7 stream so marks fire promptly
                        if marks[pr] is not None:
                            add_dep_helper(prep.ins, marks[pr], sync=False,
                                           reason="marks first")
                        ab_next.append(t)
                        gsems_next.append(gsem)

                # s=0 -> subnet2 (input x2 = ab[4:8], writes y1 into ab[0:4])
                # s=1 -> subnet1 (input y1 = ab[0:4], writes y2 into ab[4:8])
                for s in range(2):
                    bofs = (k * 2 + s) * 12
                    inp_lo = 4 * (1 - s)    # kt base of subnet input in ab
                    xm_lo = 4 * s           # kt base of mult operand in ab
                    for pair in range(NPAIR):
                        A = aepool.tile([P, KT_S, PW], F16, tag="A")
                        E = aepool.tile([P, KT_S, PW], F16, tag="E")
                        h = hpool.tile([P, KT_S, PW], F16, tag="h")
                        # mm1 per chunk (N=256; cond k-tiles first so the
                        # accumulation can start before the gather lands)
                        for half in range(2):
                            off = half * BC
                            cs = pair * PW + off
                            for mt in range(KT_S):
                                ph = pspool.tile([P, BC], F32, tag="ps")
                                kts = list(range(KT_S, KT_X)) + list(range(KT_S))
                                for i, kt in enumerate(kts):
                                    if kt < KT_S:
                                        rhs = ab[pair][:, inp_lo + kt,
                                                       off:off + BC]
                                    else:
                                        rhs = ct[:, kt - KT_S, cs:cs + BC]
                                    lo = kt * 512 + mt * P
                                    mm = nc.tensor.matmul(
                                        ph[:], w_t[s][:, lo:lo + P], rhs,
                                        start=(i == 0), stop=(i == KT_X - 1))
                                    if (s == 0 and i == KT_S
                                            and marks[pair] is not None):
                                        add_dep_helper(
                                            mm.ins, marks[pair], sync=True,
                                            reason="gather data landed")
                                nc.vector.tensor_scalar(
                                    out=h[:, mt, off:off + BC], in0=ph[:],
                                    scalar1=bsb[:, bofs + mt:bofs + mt + 1],
                                    scalar2=0.0, op0=ALU.add, op1=ALU.max)
                        # mm2 s-half at N=512 -> atan -> A
                        for mt in range(KT_S):
                            pr2 = pspool.tile([P, PW], F32, tag="ps")
                            for kt in range(KT_S):
                                lo = 4096 + kt * 1024 + mt * P
                                nc.tensor.matmul(
                                    pr2[:], w_t[s][:, lo:lo + P], h[:, kt, :],
                                    start=(kt == 0), stop=(kt == KT_S - 1))
                            nc.scalar.activation(
                                A[:, mt, :], pr2[:], AF.Arctan,
                                bias=bsb[:, bofs + 4 + mt:bofs + 5 + mt],
                                scale=1.0)
                        # exp for the whole pair (one table switch in)
                        nc.scalar.activation(E[:], A[:], AF.Exp,
                                             bias=z0[:, :1], scale=ATAN_SCALE)
                        # tmp = E * x_mult for the whole pair
                        tmp = aepool.tile([P, KT_S, PW], F16, tag="tmp")
                        mul = nc.vector.tensor_mul(
                            tmp[:], E[:], ab[pair][:, xm_lo:xm_lo + KT_S, :])
                        if marks[pair] is not None:
                            add_dep_helper(mul.ins, marks[pair], sync=True,
                                           reason="gather data landed")
                        # mm2 t-half at N=512 + y update
                        for mt in range(KT_S, 2 * KT_S):
                            pr2 = pspool.tile([P, PW], F32, tag="ps")
                            for kt in range(KT_S):
                                lo = 4096 + kt * 1024 + mt * P
                                nc.tensor.matmul(
                                    pr2[:], w_t[s][:, lo:lo + P], h[:, kt, :],
                                    start=(kt == 0), stop=(kt == KT_S - 1))
                            j = mt - KT_S
                            nc.vector.scalar_tensor_tensor(
                                out=ab[pair][:, xm_lo + j, :], in0=pr2[:],
                                scalar=bsb[:, bofs + 4 + mt:bofs + 5 + mt],
                                in1=tmp[:, j, :],
                                op0=ALU.add, op1=ALU.add)
                        # jac partition-sums over the pair (N=512)
                        pcs = pair * PW
                        for mt in range(KT_S):
                            nc.tensor.matmul(
                                jac_ps[:, pcs:pcs + PW], ones_t[:],
                                A[:, mt, :], start=not jac_started[pair],
                                stop=False, skip_group_check=True)
                            jac_started[pair] = True
                        if s == 1:
                            nc.sync.dma_start(pair_T(dst, pair), ab[pair][:])
                            if k + 1 < L:
                                b1 = brgpool.tile([1, 64], F16, tag="brg1")
                                nc.sync.dma_start(b1[:], dst[pair][0:1, 0:64])
                                b2 = brgpool.tile([1, 64], F16, tag="brg2")
                                gop = nc.gpsimd.tensor_copy(b2[:], b1[:])
                                trig = nc.gpsimd.trigger_dma(count=None,
                                                             queue_num=pair)
                                add_dep_helper(trig.ins, gop.ins, sync=False,
                                               reason="after bridge read")
                                wt_i = nc.gpsimd.wait_ge(gsems_next[pair], 16)
                                add_dep_helper(wt_i.ins, trig.ins, sync=False,
                                               reason="after trigger")
                                flag = brgpool.tile([1, 16], F16, tag="flag")
                                mark = nc.gpsimd.memset(flag[:], 0.0)
                                add_dep_helper(mark.ins, wt_i.ins, sync=False,
                                               reason="after wait")
                                marks_next[pair] = mark.ins

                ab = ab_next
                marks = marks_next

            jac_sb = cpool.tile([1, BS], F32, tag="jacsb")
            nc.scalar.mul(jac_sb[:], jac_ps[:], ATAN_SCALE)
            nc.sync.dma_start(out_j[:], jac_sb[:])

    nc.compile()
    return nc


def get_nc():
    if "nc" not in _NC_CACHE:
        _NC_CACHE["nc"] = build()
    return _NC_CACHE["nc"]


def _pack_weights(s1_w1, s1_b1, s1_w2, s1_b2, s2_w1, s2_b1, s2_w2, s2_b2):
    wts = np.empty((L, 2, P, 8192), dtype=np.float16)
    bias = np.empty((P, L * 2 * 12), dtype=np.float32)
    # s index 0 -> subnet2 (runs first), 1 -> subnet1
    for k in range(L):
        for s, (w1, b1, w2, b2) in enumerate(
            ((s2_w1, s2_b1, s2_w2, s2_b2), (s1_w1, s1_b1, s1_w2, s1_b2))
        ):
            wts[k, s, :, :4096] = (
                w1[k].reshape(8, P, 512).transpose(1, 0, 2).reshape(P, 4096))
            wts[k, s, :, 4096:] = (
                w2[k].reshape(4, P, 1024).transpose(1, 0, 2).reshape(P, 4096))
            bofs = (k * 2 + s) * 12
            bias[:, bofs:bofs + 4] = b1[k].reshape(4, P).T
            bias[:, bofs + 4:bofs + 12] = b2[k].reshape(8, P).T
    return wts, bias


def _run(inputs, trace=False):
    x = np.asarray(inputs["x"], dtype=np.float32)
    cond = np.asarray(inputs["cond"], dtype=np.float32)
    perms = np.asarray(inputs["perms"]).astype(np.int64)
    wts, bias = _pack_weights(
        np.asarray(inputs["s1_w1"], np.float32), np.asarray(inputs["s1_b1"], np.float32),
        np.asarray(inputs["s1_w2"], np.float32), np.asarray(inputs["s1_b2"], np.float32),
        np.asarray(inputs["s2_w1"], np.float32), np.asarray(inputs["s2_b1"], np.float32),
        np.asarray(inputs["s2_w2"], np.float32), np.asarray(inputs["s2_b2"], np.float32))

    # gather index tiles: 16-partition wrap, replicated across the 8 Q7 cores
    gidx = np.empty((P, L * 64), dtype=np.int16)
    for k in range(L):
        blk = np.zeros((16, 64), dtype=np.int16)
        pk = perms[k]
        for i in range(BS):
            blk[i % 16, i // 16] = pk[i]
        gidx[:, k * 64:(k + 1) * 64] = np.tile(blk, (8, 1))
    ones = np.ones((P, 1), dtype=np.float16)

    in_maps = []
    for ci in range(NCORE):
        xs = x[ci * BS:(ci + 1) * BS].T.astype(np.float16)          # [D, BS]
        xs = np.ascontiguousarray(
            xs.reshape(D, NPAIR, PW).transpose(1, 0, 2))            # [NPAIR, D, PW]
        cs = np.ascontiguousarray(cond[ci * BS:(ci + 1) * BS].T).astype(np.float16)
        in_maps.append(dict(x=xs, cond=cs, wts=wts, bias=bias,
                            gidx=gidx, ones=ones))

    nc = get_nc()
    res = run_bass_kernel_spmd(nc, in_maps, core_ids=list(range(NCORE)),
                               trace=trace)

    x_out = np.empty((B, D), dtype=np.float32)
    jac_out = np.empty((B,), dtype=np.float32)
    for ci in range(NCORE):
        oc = res.results[ci]["out"]                                 # [NPAIR, D, PW]
        x_out[ci * BS:(ci + 1) * BS] = (
            oc.transpose(1, 0, 2).reshape(D, BS).T.astype(np.float32))
        jac_out[ci * BS:(ci + 1) * BS] = res.results[ci]["jac"][0]
    return (x_out, jac_out), res


def kernel(**inputs):
    out, _ = _run(inputs, trace=False)
    return out


def kernel_traced(**inputs):
    return _run(inputs, trace=True)
